# revision 1
# baseline (speedup 1.0000x reference)
"""EquivariantGraphConvolution (EGNN layer) on 8 Trainium2 NeuronCores.

Strategy
--------
Nodes are range-partitioned across the 8 cores (6250 nodes each); every edge is
owned by the core that owns its *start* node, so the per-start segment sums are
core-local and no collective is needed.  Per core, edges are bucketed by
(128-node start block, end<25000) cells and padded to a static layout of
49 blocks x (12+12) chunks x 128 edges.

Per-node first-layer partial products P1 = nf @ W_e1[0:64] and
P2 = nf @ W_e1[64:128] are computed on device into bf16 DRAM tables; per-edge
values are fetched with transposed dma_gather (features-on-partitions).  The
edge MLP runs feature-major with 2-group block-diagonal matmuls; the
feature->edge-major flip and the segment sum are per-128-edge-chunk matmuls
(one-hot scatter).  Node updates (velocity/node MLPs, coordinate update) run
on-chip afterwards; the host only shards, sorts, and concatenates.
"""
import sys
sys.path.insert(0, "/opt/trn_rl_repo")
import contextlib
import numpy as np

import concourse.bass as bass
import concourse.bacc as bacc
import concourse.mybir as mybir
import concourse.tile as tile
from concourse.bass_utils import run_bass_kernel_spmd

f32 = mybir.dt.float32
bf16 = mybir.dt.bfloat16
i16 = mybir.dt.int16
i32 = mybir.dt.int32
AF = mybir.ActivationFunctionType
OP = mybir.AluOpType

# ---- problem constants (hardcoded per contract) ----
N = 50000
E = 1_000_000
H = 64
EF = 16
NCORES = 8
NPC = N // NCORES          # 6250 nodes per core
NBLK = 49                  # 128-node blocks per core (49*128 = 6272 >= 6250)
NPAD = NBLK * 128          # 6272
TSPLIT = 25000             # end-index split so gather indices fit int16
CPR = 12                   # chunks per (block, range) cell
CELL = CPR * 128           # 1536 edge slots per cell
BLKE = 2 * CELL            # 3072 edge slots per block
ES = NBLK * BLKE           # 150528 edge slots per core
NSUP = 3                   # supertiles per block (512-edge groups per cell)
NST = NBLK * NSUP          # 147 supertiles
NFULL = 50176              # 50000 padded to 128*392

_cache = {}


def _f(x):
    return np.ascontiguousarray(x, np.float32)


def _prep_weights(inp):
    """Small weight/constant tensors, identical on all cores."""
    W_e1 = _f(inp["W_e1"])           # [145, 64]
    w = {}
    w["w1aw"] = np.concatenate([W_e1[0:64], ], 1)
    w["w1aw"] = np.concatenate([W_e1[0:64], np.zeros((64, 64), np.float32)], 1)  # [64,128]
    w["w1bw"] = np.concatenate([W_e1[64:128], np.zeros((64, 64), np.float32)], 1)
    wef = np.zeros((34, 128), np.float32)
    wef[0:16, 0:64] = W_e1[129:145]
    wef[16:17, 0:64] = W_e1[128:129]
    wef[17:33, 64:128] = W_e1[129:145]
    wef[33:34, 64:128] = W_e1[128:129]
    w["wefcdn"] = wef
    inj = np.zeros((64, 128), np.float32)
    inj[:, 0:64] = np.eye(64)
    w["injA"] = inj
    injb = np.zeros((64, 128), np.float32)
    injb[:, 64:128] = np.eye(64)
    w["injB"] = injb
    W_e2 = _f(inp["W_e2"]); W_c1 = _f(inp["W_c1"])
    bd = np.zeros((128, 128), np.float32)
    bd[0:64, 0:64] = W_e2; bd[64:128, 64:128] = W_e2
    w["wde2"] = bd
    bc = np.zeros((128, 128), np.float32)
    bc[0:64, 0:64] = W_c1; bc[64:128, 64:128] = W_c1
    w["wdc1"] = bc
    W_i = _f(inp["W_i"]); W_c2 = _f(inp["W_c2"])
    ra = np.zeros((128, 65), np.float32)
    ra[0:64, 0:64] = np.eye(64); ra[0:64, 64:65] = W_i
    ra[64:128, 0:64] = np.eye(64); ra[64:128, 64:65] = W_i
    w["raw2"] = ra
    wc2r = np.zeros((128, 1), np.float32)
    wc2r[0:64] = W_c2; wc2r[64:128] = W_c2
    w["wc2r"] = wc2r
    w["wn1"] = _f(inp["W_n1"])       # [128, 64]
    w["wn2"] = _f(inp["W_n2"])       # [64, 64]
    w["wv1"] = _f(inp["W_v1"])       # [64, 64]
    w["wv2"] = _f(inp["W_v2"])       # [64, 1]
    w["eye64"] = np.eye(64, dtype=np.float32)

    def col(v, n):
        return np.tile(_f(v).reshape(-1, 1), (n // len(np.atleast_1d(v)), 1)) \
            if False else None
    b_e1 = _f(inp["b_e1"]); b_e2 = _f(inp["b_e2"]); b_c1 = _f(inp["b_c1"])
    w["be1s"] = np.concatenate([b_e1, b_e1]).reshape(128, 1)
    w["be2s"] = np.concatenate([b_e2, b_e2]).reshape(128, 1)
    w["bc1s"] = np.concatenate([b_c1, b_c1]).reshape(128, 1)
    w["bih"] = np.full((128, 1), 0.5 * float(np.asarray(inp["b_i"]).ravel()[0]), np.float32)
    w["bn1c"] = _f(inp["b_n1"]).reshape(64, 1)
    w["bn2c"] = _f(inp["b_n2"]).reshape(64, 1)
    w["bv1c"] = _f(inp["b_v1"]).reshape(64, 1)
    w["bv2c"] = np.full((1, 1), float(np.asarray(inp["b_v2"]).ravel()[0]), np.float32)
    return w


def _wrap16(idx_slots):
    """[n] int16 -> [128, n/16]: index i at (i%16, i//16), replicated x8."""
    n = idx_slots.shape[0]
    base = idx_slots.reshape(n // 16, 16).T          # [16, n/16]
    return np.tile(base, (8, 1)).astype(np.int16)    # [128, n/16]


def _prep_core(c, start, end, ef, nfi, cd_all, cdn_all, invcnt_all):
    lo, hi = c * NPC, (c + 1) * NPC
    sel = (start >= lo) & (start < hi)
    eo = np.nonzero(sel)[0]
    s_loc = (start[eo] - lo).astype(np.int64)
    e_g = end[eo].astype(np.int64)
    blk = s_loc >> 7
    rbit = (e_g >= TSPLIT).astype(np.int64)
    cellid = blk * 2 + rbit
    order = np.argsort(cellid, kind="stable")
    eo = eo[order]; s_loc = s_loc[order]; e_g = e_g[order]
    blk = blk[order]; rbit = rbit[order]; cellid = cellid[order]
    counts = np.bincount(cellid, minlength=2 * NBLK)
    if counts.max() > CELL:
        raise RuntimeError(f"cell overflow: {counts.max()} > {CELL}")
    starts = np.zeros(2 * NBLK, np.int64)
    starts[1:] = np.cumsum(counts)[:-1]
    within = np.arange(len(eo)) - starts[cellid]
    slots = blk * BLKE + rbit * CELL + within + (cellid % 2) * 0
    slots = blk * BLKE + rbit * CELL + within

    g1 = np.zeros(ES, np.int64); g2 = np.zeros(ES, np.int64)
    lid = np.full(ES, -1.0, np.float32)
    cds = np.zeros((ES, 3), np.float32)
    cdns = np.zeros(ES, np.float32)
    efs = np.zeros((ES, EF), np.float32)
    g1[slots] = s_loc
    g2[slots] = e_g - rbit * TSPLIT
    lid[slots] = (s_loc & 127).astype(np.float32)
    cds[slots] = cd_all[eo]
    cdns[slots] = cdn_all[eo]
    efs[slots] = ef[eo]

    d = {}
    a = g1.reshape(NBLK, BLKE)
    d["g1i"] = np.stack([_wrap16(a[b]) for b in range(NBLK)])          # [NBLK,128,192]
    a2 = g2.reshape(NBLK, 2, CELL)
    d["g2ilo"] = np.stack([_wrap16(a2[b, 0]) for b in range(NBLK)])    # [NBLK,128,96]
    d["g2ihi"] = np.stack([_wrap16(a2[b, 1]) for b in range(NBLK)])
    d["lidx"] = lid.reshape(NBLK, 24, 128).transpose(0, 2, 1).copy()   # [NBLK,128,24]
    d["cdem"] = cds.reshape(NBLK, 24, 128, 3).transpose(0, 2, 1, 3).copy()  # [NBLK,128,24,3]
    efr = efs.reshape(NBLK, 2, NSUP, 512, EF)
    cdnr = cdns.reshape(NBLK, 2, NSUP, 512)
    eA = np.concatenate([efr[:, 0].transpose(0, 1, 3, 2),
                         cdnr[:, 0][:, :, None, :]], axis=2)           # [NBLK,3,17,512]
    eB = np.concatenate([efr[:, 1].transpose(0, 1, 3, 2),
                         cdnr[:, 1][:, :, None, :]], axis=2)
    d["efcdn"] = np.concatenate([eA, eB], axis=2).reshape(NST, 34, 512).copy()

    nm = np.zeros((NPAD, 70), np.float32)
    nm[0:NPC] = nfi[lo:hi]
    d["node_nm"] = nm.reshape(NBLK, 128, 70).transpose(1, 0, 2).reshape(128, NBLK * 70).copy()
    ic = np.ones(NPAD, np.float32)
    ic[0:NPC] = invcnt_all[lo:hi]
    d["invcnt"] = ic.reshape(NBLK, 128).T.copy()                        # [128, NBLK]
    nl = np.zeros((64, NPAD), np.float32)
    nl[:, 0:NPC] = nfi[lo:hi, 6:70].T
    d["nfT_local"] = nl
    return d


def _build_program():
    import os
    STAGE = int(os.environ.get("EGNN_STAGE", "5"))
    nc = bacc.Bacc("TRN2", target_bir_lowering=False, debug=False,
                   enable_asserts=False, num_devices=NCORES)

    def din(name, shape, dt=f32):
        return nc.dram_tensor(name, list(shape), dt, kind="ExternalInput").ap()

    nfT_full = din("nfT_full", [64, NFULL])
    g1i = din("g1i", [NBLK, 128, 192], i16)
    g2ilo = din("g2ilo", [NBLK, 128, 96], i16)
    g2ihi = din("g2ihi", [NBLK, 128, 96], i16)
    lidx_d = din("lidx", [NBLK, 128, 24])
    cdem_d = din("cdem", [NBLK, 128, 24, 3])
    efcdn_d = din("efcdn", [NST, 34, 512])
    invcnt_d = din("invcnt", [128, NBLK])
    node_nm_d = din("node_nm", [128, NBLK * 70])
    nfT_loc_d = din("nfT_local", [64, NPAD])
    wnames = ["w1aw", "w1bw", "wefcdn", "injA", "injB", "wde2", "wdc1",
              "raw2", "wc2r", "wn1", "wn2", "wv1", "wv2", "eye64",
              "be1s", "be2s", "bc1s", "bih", "bn1c", "bn2c", "bv1c", "bv2c"]
    wshapes = {"w1aw": [64, 128], "w1bw": [64, 128], "wefcdn": [34, 128],
               "injA": [64, 128], "injB": [64, 128], "wde2": [128, 128],
               "wdc1": [128, 128], "raw2": [128, 65], "wc2r": [128, 1],
               "wn1": [128, 64], "wn2": [64, 64], "wv1": [64, 64],
               "wv2": [64, 1], "eye64": [64, 64], "be1s": [128, 1],
               "be2s": [128, 1], "bc1s": [128, 1], "bih": [128, 1],
               "bn1c": [64, 1], "bn2c": [64, 1], "bv1c": [64, 1], "bv2c": [1, 1]}
    wd = {n: din(n, wshapes[n]) for n in wnames}
    out_d = nc.dram_tensor("out", [NPAD, 70], f32, kind="ExternalOutput").ap()
    T1 = nc.dram_tensor("T1", [NPAD, 128], bf16).ap()
    T2 = nc.dram_tensor("T2", [NFULL, 128], bf16).ap()

    BF_W = ("injA", "injB", "wdc1", "raw2", "wc2r")

    with tile.TileContext(nc) as tc, contextlib.ExitStack() as ctx:
        wpool = ctx.enter_context(tc.tile_pool(name="w", bufs=1))
        wt = {}
        for n in wnames:
            dt = bf16 if n in BF_W else f32
            t = wpool.tile(wshapes[n], dt, name=f"wt_{n}")
            if dt == f32:
                nc.sync.dma_start(t[:], wd[n][:])
            else:
                tf = wpool.tile(wshapes[n], f32, name=f"wtf_{n}")
                nc.sync.dma_start(tf[:], wd[n][:])
                nc.vector.tensor_copy(t[:], tf[:])
            wt[n] = t
        iota32 = wpool.tile([128, 128], i32, name="iota32")
        nc.gpsimd.iota(iota32[:], pattern=[[1, 128]], base=0, channel_multiplier=0)
        iota = wpool.tile([128, 128], f32, name="iota")
        nc.vector.tensor_copy(iota[:], iota32[:])
        node_nm = wpool.tile([128, NBLK * 70], f32, name="node_nm")
        nc.sync.dma_start(node_nm[:], node_nm_d[:])
        invcnt = wpool.tile([128, NBLK], f32, name="invcnt")
        nc.sync.dma_start(invcnt[:], invcnt_d[:])
        nfT_loc = wpool.tile([64, NPAD], f32, name="nfT_loc")
        nc.sync.dma_start(nfT_loc[:], nfT_loc_d[:])
        vscale = wpool.tile([128, NBLK], f32, name="vscale")
        aggsb = wpool.tile([128, NPAD], f32, name="aggsb")

        # ---------- Phase A: P-tables ----------
        with tc.tile_pool(name="pa", bufs=3) as pa, \
             tc.tile_pool(name="pap", bufs=3, space="PSUM") as pap:
            for j in range(NFULL // 512):
                nft = pa.tile([64, 512], f32, name=f"nft{j}", tag="nft")
                nc.sync.dma_start(nft[:], nfT_full[:, j * 512:(j + 1) * 512])
                for k in range(4):
                    ps = pap.tile([128, 128], f32, name=f"aps{j}_{k}", tag="aps")
                    nc.tensor.matmul(ps[:], nft[:, k * 128:(k + 1) * 128], wt["w1bw"][:])
                    row = pa.tile([128, 128], bf16, name=f"arow{j}_{k}", tag="arow")
                    nc.any.tensor_copy(row[:], ps[:])
                    r0 = j * 512 + k * 128
                    nc.sync.dma_start(T2[r0:r0 + 128, :], row[:])
            for k in range(NBLK):
                ps = pap.tile([128, 128], f32, name=f"bps{k}", tag="aps")
                nc.tensor.matmul(ps[:], nfT_loc[:, k * 128:(k + 1) * 128], wt["w1aw"][:])
                row = pa.tile([128, 128], bf16, name=f"brow{k}", tag="arow")
                nc.any.tensor_copy(row[:], ps[:])
                nc.sync.dma_start(T1[k * 128:(k + 1) * 128, :], row[:])

        # ---------- Phase B: velocity MLP -> vscale [128, NBLK] ----------
        with tc.tile_pool(name="pb", bufs=2) as pb, \
             tc.tile_pool(name="pbp", bufs=2, space="PSUM") as pbp:
            tiles = [(j * 512, 512) for j in range(NPAD // 512)]
            if NPAD % 512:
                tiles.append((NPAD // 512 * 512, NPAD % 512))
            for (o, L) in tiles:
                vps = pbp.tile([64, L], f32, name=f"vps{o}", tag="vps")
                nc.tensor.matmul(vps[:], wt["wv1"][:], nfT_loc[:, o:o + L])
                vh = pb.tile([64, L], f32, name=f"vh{o}", tag="vh")
                nc.scalar.activation(vh[:], vps[:], AF.Silu, bias=wt["bv1c"][:])
                sps = pbp.tile([1, L], f32, name=f"sps{o}", tag="sps")
                nc.tensor.matmul(sps[:], wt["wv2"][:], vh[:])
                vsc = pb.tile([1, L], f32, name=f"vsc{o}", tag="vsc")
                nc.scalar.activation(vsc[:], sps[:], AF.Identity, bias=wt["bv2c"][:])
                for k in range(L // 128):
                    tp = pbp.tile([128, 1], f32, name=f"tp{o}_{k}", tag="tp")
                    nc.tensor.transpose(tp[:], vsc[:, k * 128:(k + 1) * 128],
                                        wt["eye64"][0:1, 0:1])
                    nc.vector.tensor_copy(vscale[:, o // 128 + k:o // 128 + k + 1], tp[:])

        # ---------- Edge sweep ----------
        if STAGE >= 2:
            _edge_sweep(nc, tc, STAGE, wt, iota, g1i, g2ilo, g2ihi, lidx_d,
                        cdem_d, efcdn_d, T1, T2, aggsb)

        # ---------- Phase C: node update + output ----------
        if STAGE >= 5:
            _phase_c(nc, tc, wt, nfT_loc, aggsb, node_nm, invcnt, vscale, out_d)

    nc.compile()
    return nc


def _edge_sweep(nc, tc, STAGE, wt, iota, g1i, g2ilo, g2ihi, lidx_d, cdem_d,
                efcdn_d, T1, T2, aggsb):
    with tc.tile_pool(name="pg", bufs=2) as pg, \
             tc.tile_pool(name="pe", bufs=2) as pe, \
             tc.tile_pool(name="pch", bufs=3) as pch, \
             tc.tile_pool(name="px", bufs=2, space="PSUM") as px, \
             tc.tile_pool(name="pm", bufs=2, space="PSUM") as pm, \
             tc.tile_pool(name="pst", bufs=2, space="PSUM") as pst, \
             tc.tile_pool(name="pagg", bufs=2, space="PSUM") as pagg:
            for b in range(NBLK):
                g1x = pg.tile([128, 192], i16, name=f"g1x{b}", tag="g1x")
                nc.sync.dma_start(g1x[:], g1i[b])
                g2xl = pg.tile([128, 96], i16, name=f"g2xl{b}", tag="g2xl")
                nc.sync.dma_start(g2xl[:], g2ilo[b])
                g2xh = pg.tile([128, 96], i16, name=f"g2xh{b}", tag="g2xh")
                nc.sync.dma_start(g2xh[:], g2ihi[b])
                g1t = pg.tile([128, BLKE], bf16, name=f"g1t{b}", tag="g1t")
                nc.gpsimd.dma_gather(
                    out_ap=g1t[:].rearrange("p (o n) -> p o n", o=1),
                    in_ap=T1[:], idxs_ap=g1x[:],
                    num_idxs=BLKE, num_idxs_reg=BLKE, elem_size=128, transpose=True,
                    single_packet=False)
                g2lt = pg.tile([128, CELL], bf16, name=f"g2lt{b}", tag="g2lt")
                nc.gpsimd.dma_gather(
                    out_ap=g2lt[:].rearrange("p (o n) -> p o n", o=1),
                    in_ap=T2[:], idxs_ap=g2xl[:],
                    num_idxs=CELL, num_idxs_reg=CELL, elem_size=128, transpose=True,
                    single_packet=False)
                g2ht = pg.tile([128, CELL], bf16, name=f"g2ht{b}", tag="g2ht")
                nc.gpsimd.dma_gather(
                    out_ap=g2ht[:].rearrange("p (o n) -> p o n", o=1),
                    in_ap=T2[TSPLIT:NFULL, :], idxs_ap=g2xh[:],
                    num_idxs=CELL, num_idxs_reg=CELL, elem_size=128, transpose=True,
                    single_packet=False)
                lidt = pg.tile([128, 24], f32, name=f"lidt{b}", tag="lidt")
                nc.sync.dma_start(lidt[:], lidx_d[b])
                cdt = pg.tile([128, 24, 3], f32, name=f"cdt{b}", tag="cdt")
                nc.sync.dma_start(cdt[:], cdem_d[b])

                if STAGE == 2:
                    nc.any.tensor_copy(aggsb[:, b * 128:(b + 1) * 128],
                                       g1t[:, 0:128])
                    continue
                aggT = pagg.tile([128, 128], f32, name=f"aggT{b}", tag="aggT")
                for s in range(NSUP):
                    eft = pe.tile([34, 512], f32, name=f"eft{b}_{s}", tag="eft")
                    nc.sync.dma_start(eft[:], efcdn_d[b * NSUP + s])
                    sl = slice(s * 512, (s + 1) * 512)
                    slh = slice(CELL + s * 512, CELL + (s + 1) * 512)
                    sA = pe.tile([64, 512], bf16, name=f"sA{b}_{s}", tag="sA")
                    nc.vector.tensor_tensor(sA[:], g1t[0:64, sl], g2lt[0:64, sl], OP.add)
                    sB = pe.tile([64, 512], bf16, name=f"sB{b}_{s}", tag="sB")
                    nc.vector.tensor_tensor(sB[:], g1t[0:64, slh], g2ht[0:64, sl], OP.add)
                    x1 = px.tile([128, 512], f32, name=f"x1{b}_{s}", tag="x1")
                    nc.tensor.matmul(x1[:], wt["injA"][:], sA[:], start=True, stop=False)
                    nc.tensor.matmul(x1[:], wt["injB"][:], sB[:], start=False, stop=False)
                    nc.tensor.matmul(x1[:], wt["wefcdn"][:], eft[:], start=False, stop=True)
                    h1 = pe.tile([128, 512], f32, name=f"h1{b}_{s}", tag="h1")
                    nc.scalar.activation(h1[:], x1[:], AF.Silu, bias=wt["be1s"][:])
                    mp = pm.tile([128, 512], f32, name=f"mp{b}_{s}", tag="mm2")
                    nc.tensor.matmul(mp[:], wt["wde2"][:], h1[:])
                    msgT = pe.tile([128, 512], bf16, name=f"msgT{b}_{s}", tag="msgT")
                    nc.scalar.activation(msgT[:], mp[:], AF.Silu, bias=wt["be2s"][:])
                    cp = pm.tile([128, 512], f32, name=f"cp{b}_{s}", tag="mm2")
                    nc.tensor.matmul(cp[:], wt["wdc1"][:], msgT[:])
                    chT = pe.tile([128, 512], bf16, name=f"chT{b}_{s}", tag="chT")
                    nc.scalar.activation(chT[:], cp[:], AF.Silu, bias=wt["bc1s"][:])
                    if STAGE == 3:
                        if s == 0:
                            nc.any.tensor_copy(
                                aggsb[:, b * 128:(b + 1) * 128], chT[:, 0:128])
                        continue
                    for g in range(2):
                        rows = slice(g * 64, g * 64 + 64)
                        jb = 12 * g + s * 4
                        st = pst.tile([128, 4, 66], f32, name=f"st{b}_{s}_{g}", tag="st")
                        for c4 in range(4):
                            cc = slice(c4 * 128, (c4 + 1) * 128)
                            nc.tensor.matmul(st[:, c4, 0:65], msgT[rows, cc],
                                             wt["raw2"][rows, :], start=True, stop=True)
                            nc.tensor.matmul(st[:, c4, 65:66], chT[rows, cc],
                                             wt["wc2r"][rows, :], start=True, stop=True)
                        tnh = pch.tile([128, 4], f32, name=f"tnh{b}_{s}_{g}", tag="tnh")
                        nc.scalar.activation(tnh[:], st[:, :, 64:65].squeeze(2),
                                             AF.Tanh, bias=wt["bih"][:], scale=0.5)
                        gate = pch.tile([128, 4], f32, name=f"gt{b}_{s}_{g}", tag="gate")
                        nc.vector.tensor_scalar(out=gate[:], in0=tnh[:], scalar1=1.0,
                                                scalar2=0.5, op0=OP.add, op1=OP.mult)
                        rg = pch.tile([128, 4, 64], f32, name=f"rg{b}_{s}_{g}", tag="rg")
                        nc.vector.tensor_tensor(
                            rg[:], st[:, :, 0:64],
                            gate[:].unsqueeze(2).broadcast_to([128, 4, 64]), OP.mult)
                        rc = pch.tile([128, 4, 3], f32, name=f"rc{b}_{s}_{g}", tag="rc")
                        nc.vector.tensor_tensor(
                            rc[:], cdt[:, jb:jb + 4, :],
                            st[:, :, 65:66].broadcast_to([128, 4, 3]), OP.mult)
                        oht = pch.tile([128, 4, 128], f32, name=f"oh{b}_{s}_{g}", tag="oh")
                        nc.vector.tensor_tensor(
                            oht[:], iota[:].unsqueeze(1).broadcast_to([128, 4, 128]),
                            lidt[:, jb:jb + 4].unsqueeze(2).broadcast_to([128, 4, 128]),
                            OP.is_equal)
                        for c4 in range(4):
                            first = (s == 0 and g == 0 and c4 == 0)
                            last = (s == NSUP - 1 and g == 1 and c4 == 3)
                            nc.tensor.matmul(aggT[0:3, :], rc[:, c4, :], oht[:, c4, :],
                                             start=first, stop=last,
                                             tile_position=(0, 0), skip_group_check=True)
                            nc.tensor.matmul(aggT[64:128, :], rg[:, c4, :], oht[:, c4, :],
                                             start=first, stop=last,
                                             tile_position=(0, 64), skip_group_check=True)
                if STAGE == 3:
                    continue
                nc.any.tensor_copy(aggsb[:, b * 128:(b + 1) * 128], aggT[:])


def _phase_c(nc, tc, wt, nfT_loc, aggsb, node_nm, invcnt, vscale, out_d):
    with tc.tile_pool(name="pc", bufs=3) as pc, \
             tc.tile_pool(name="pcp", bufs=2, space="PSUM") as pcp:
            for b in range(NBLK):
                cols = slice(b * 128, (b + 1) * 128)
                xnT = pc.tile([128, 128], f32, name=f"xnT{b}", tag="xnT")
                nc.vector.tensor_copy(xnT[0:64, :], nfT_loc[:, cols])
                nc.vector.tensor_copy(xnT[64:128, :], aggsb[64:128, cols])
                n1 = pcp.tile([64, 128], f32, name=f"n1{b}", tag="n1")
                nc.tensor.matmul(n1[:], wt["wn1"][:], xnT[:])
                hn = pc.tile([64, 128], f32, name=f"hn{b}", tag="hn")
                nc.scalar.activation(hn[:], n1[:], AF.Silu, bias=wt["bn1c"][:])
                n2 = pcp.tile([64, 128], f32, name=f"n2{b}", tag="n2")
                nc.tensor.matmul(n2[:], wt["wn2"][:], hn[:])
                hn2 = pc.tile([64, 128], f32, name=f"hn2{b}", tag="hn2")
                nc.scalar.activation(hn2[:], n2[:], AF.Identity, bias=wt["bn2c"][:])
                ndel = pcp.tile([128, 64], f32, name=f"ndel{b}", tag="ndel")
                nc.tensor.transpose(ndel[:], hn2[:], wt["eye64"][:])
                ctp = pcp.tile([128, 3], f32, name=f"ctp{b}", tag="ctp")
                nc.tensor.transpose(ctp[:], aggsb[0:3, cols], wt["eye64"][0:3, 0:3])
                nmb = node_nm[:, b * 70:(b + 1) * 70]
                t1 = pc.tile([128, 3], f32, name=f"t1{b}", tag="t1")
                nc.vector.tensor_scalar(out=t1[:], in0=ctp[:],
                                        scalar1=invcnt[:, b:b + 1], scalar2=None,
                                        op0=OP.mult)
                t2 = pc.tile([128, 3], f32, name=f"t2{b}", tag="t2")
                nc.vector.tensor_scalar(out=t2[:], in0=nmb[:, 3:6],
                                        scalar1=vscale[:, b:b + 1], scalar2=None,
                                        op0=OP.mult)
                t3 = pc.tile([128, 3], f32, name=f"t3{b}", tag="t3")
                nc.vector.tensor_tensor(t3[:], t1[:], t2[:], OP.add)
                ot = pc.tile([128, 70], f32, name=f"ot{b}", tag="ot")
                nc.vector.tensor_tensor(ot[:, 0:3], t3[:], nmb[:, 0:3], OP.add)
                nc.vector.tensor_copy(ot[:, 3:6], nmb[:, 3:6])
                nc.vector.tensor_tensor(ot[:, 6:70], nmb[:, 6:70], ndel[:], OP.add)
                nc.sync.dma_start(out_d[b * 128:(b + 1) * 128, :], ot[:])


def kernel(**inputs):
    ei = np.asarray(inputs["edge_indices"])
    start = ei[0].astype(np.int64)
    end = ei[1].astype(np.int64)
    ef = _f(inputs["edge_features"])
    nfi = _f(inputs["node_features_input"])
    coords = nfi[:, 0:3]
    cd_all = coords[start] - coords[end]
    cdn_all = np.sqrt((cd_all ** 2).sum(1)).astype(np.float32)
    deg = np.bincount(start, minlength=N).astype(np.float32)
    invcnt_all = (1.0 / np.maximum(deg, 1.0)).astype(np.float32)

    w = _prep_weights(inputs)
    nfT_full = np.zeros((64, NFULL), np.float32)
    nfT_full[:, 0:N] = nfi[:, 6:70].T

    in_maps = []
    for c in range(NCORES):
        d = _prep_core(c, start, end, ef, nfi, cd_all, cdn_all, invcnt_all)
        d.update(w)
        d["nfT_full"] = nfT_full
        in_maps.append(d)

    if "nc" not in _cache:
        _cache["nc"] = _build_program()
    nc = _cache["nc"]
    _cache["in_maps"] = in_maps
    res = run_bass_kernel_spmd(nc, in_maps, list(range(NCORES)))
    out = np.empty((N, 70), np.float32)
    for c in range(NCORES):
        out[c * NPC:(c + 1) * NPC] = res.results[c]["out"][0:NPC]
    return out



# revision 4
# speedup vs baseline: 4.2269x; 4.2269x over previous
"""EquivariantGraphConvolution (EGNN layer) on 8 Trainium2 NeuronCores.

Strategy (v2 — streamed, gather-free)
-------------------------------------
Nodes are range-partitioned across the 8 cores (6250 each); every edge is owned
by the core that owns its *start* node, so per-start segment sums are
core-local and no collective is needed.

Per core, edges are sorted by 128-node start block and padded per block to a
uniform CPB chunks of 128 edges.  The host pre-gathers both endpoints' node
features per edge and stages them as sequentially-streamed feature-major bf16
tensors (plus edge features / dist / coords-diff / lane ids), so the device
does ZERO indirect DMA — the edge MLP is pure dense matmul work:

  x1[128,512] = W1s_bd.T@nfs + W1e_bd.T@nfe + Wef.T@efcdn   (2 edge groups
  feature-stacked on partitions), SiLU chains for message/coords nets, a merged
  transpose+gate+coordw matmul per 128-edge chunk (K=128: msg|coord stacked),
  and a one-hot matmul segment-sum per chunk with the one-hot stationary
  (out is node-major [128,67] = 64 msg-agg + 3 coord-agg).

Node updates (velocity/node MLPs, coordinate update) run on-chip afterwards.
"""
import sys
sys.path.insert(0, "/opt/trn_rl_repo")
import contextlib
import numpy as np
import ml_dtypes

import concourse.bass as bass
import concourse.bacc as bacc
import concourse.mybir as mybir
import concourse.tile as tile
from concourse.bass_utils import run_bass_kernel_spmd

f32 = mybir.dt.float32
bf16 = mybir.dt.bfloat16
i32 = mybir.dt.int32
AF = mybir.ActivationFunctionType
OP = mybir.AluOpType
BF = ml_dtypes.bfloat16

# ---- problem constants (hardcoded per contract) ----
N = 50000
E = 1_000_000
H = 64
EF = 16
NCORES = 8
NPC = N // NCORES          # 6250 nodes per core
NBLK = 49                  # 128-node blocks per core (49*128 = 6272 >= 6250)
NPAD = NBLK * 128          # 6272

_cache = {}


def _f(x):
    return np.ascontiguousarray(x, np.float32)


def _bd(W):
    """[64,64] -> [128,128] block diagonal."""
    out = np.zeros((128, 128), np.float32)
    out[0:64, 0:64] = W
    out[64:128, 64:128] = W
    return out


def _prep_weights(inp):
    """Small weight/constant tensors, identical on all cores."""
    W_e1 = _f(inp["W_e1"])           # [145, 64]
    w = {}
    w["w1s"] = _bd(W_e1[0:64]).astype(BF)
    w["w1e"] = _bd(W_e1[64:128]).astype(BF)
    wef = np.zeros((34, 128), np.float32)
    wef[0:16, 0:64] = W_e1[129:145]
    wef[16:17, 0:64] = W_e1[128:129]
    wef[17:33, 64:128] = W_e1[129:145]
    wef[33:34, 64:128] = W_e1[128:129]
    w["wefcdn"] = wef.astype(BF)
    w["wde2"] = _bd(_f(inp["W_e2"])).astype(BF)
    w["wdc1"] = _f(inp["W_c1"]).astype(BF)          # [64, 64]
    W_i = _f(inp["W_i"]); W_c2 = _f(inp["W_c2"])
    ra = np.zeros((128, 66), np.float32)
    ra[0:64, 0:64] = np.eye(64)
    ra[0:64, 64:65] = W_i
    ra[64:128, 65:66] = W_c2
    w["raw2c"] = ra.astype(BF)
    w["wn1"] = _f(inp["W_n1"])       # [128, 64]
    w["wn2"] = _f(inp["W_n2"])       # [64, 64]
    w["wv1"] = _f(inp["W_v1"])       # [64, 64]
    w["wv2"] = _f(inp["W_v2"])       # [64, 1]
    w["eye64"] = np.eye(64, dtype=np.float32)
    w["eye128"] = np.eye(128, dtype=np.float32)
    b_e1 = _f(inp["b_e1"])
    w["be1s"] = np.concatenate([b_e1, b_e1]).reshape(128, 1)
    w["be2c"] = _f(inp["b_e2"]).reshape(64, 1)
    w["bc1c"] = _f(inp["b_c1"]).reshape(64, 1)
    w["bih"] = np.full((128, 1), 0.5 * float(np.asarray(inp["b_i"]).ravel()[0]), np.float32)
    w["bn1c"] = _f(inp["b_n1"]).reshape(64, 1)
    w["bn2c"] = _f(inp["b_n2"]).reshape(64, 1)
    w["bv1c"] = _f(inp["b_v1"]).reshape(64, 1)
    w["bv2c"] = np.full((1, 1), float(np.asarray(inp["b_v2"]).ravel()[0]), np.float32)
    return w


WSHAPES = {"w1s": [128, 128], "w1e": [128, 128], "wefcdn": [34, 128],
           "wde2": [128, 128], "wdc1": [64, 64], "raw2c": [128, 66],
           "wn1": [128, 64], "wn2": [64, 64], "wv1": [64, 64],
           "wv2": [64, 1], "eye64": [64, 64], "eye128": [128, 128],
           "be1s": [128, 1], "be2c": [64, 1], "bc1c": [64, 1],
           "bih": [128, 1], "bn1c": [64, 1], "bn2c": [64, 1],
           "bv1c": [64, 1], "bv2c": [1, 1]}
BF_W = ("w1s", "w1e", "wefcdn", "wde2", "wdc1", "raw2c")


def _prep_core(c, start, end, ef, nfi, cd_all, cdn_all, invcnt_all, CPB, S):
    """Per-core staged edge streams (sorted by start block, block-padded)."""
    NCH = S * 8
    NSLOT = NCH * 128
    lo, hi = c * NPC, (c + 1) * NPC
    sel = (start >= lo) & (start < hi)
    eo = np.nonzero(sel)[0]
    s_loc = (start[eo] - lo).astype(np.int64)
    blk = s_loc >> 7
    order = np.argsort(blk, kind="stable")
    eo = eo[order]; s_loc = s_loc[order]; blk = blk[order]
    counts = np.bincount(blk, minlength=NBLK)
    if counts.max() > CPB * 128:
        raise RuntimeError(f"block overflow: {counts.max()} > {CPB * 128}")
    starts = np.zeros(NBLK, np.int64)
    starts[1:] = np.cumsum(counts)[:-1]
    within = np.arange(len(eo)) - starts[blk]
    slots = blk * (CPB * 128) + within

    nf64 = nfi[:, 6:70]
    nfs_sl = np.zeros((NSLOT, 64), np.float32)
    nfe_sl = np.zeros((NSLOT, 64), np.float32)
    ef_sl = np.zeros((NSLOT, EF), np.float32)
    cdn_sl = np.zeros(NSLOT, np.float32)
    cd_sl = np.zeros((NSLOT, 3), np.float32)
    lid_sl = np.full(NSLOT, -1.0, np.float32)
    nfs_sl[slots] = nf64[start[eo]]
    nfe_sl[slots] = nf64[end[eo]]
    ef_sl[slots] = ef[eo]
    cdn_sl[slots] = cdn_all[eo]
    cd_sl[slots] = cd_all[eo]
    lid_sl[slots] = (s_loc & 127).astype(np.float32)

    d = {}
    # feature-major, 2 edge groups of 512 stacked on partitions
    v = nfs_sl.reshape(S, 2, 512, 64).transpose(0, 1, 3, 2)
    d["nfsT"] = np.ascontiguousarray(v.reshape(S, 128, 512)).astype(BF)
    v = nfe_sl.reshape(S, 2, 512, 64).transpose(0, 1, 3, 2)
    d["nfeT"] = np.ascontiguousarray(v.reshape(S, 128, 512)).astype(BF)
    eft = ef_sl.reshape(S, 2, 512, EF).transpose(0, 1, 3, 2)   # [S,2,16,512]
    cdnr = cdn_sl.reshape(S, 2, 512)
    d["efcdn"] = np.concatenate(
        [eft[:, 0], cdnr[:, 0][:, None, :], eft[:, 1], cdnr[:, 1][:, None, :]],
        axis=1).astype(BF)                                      # [S,34,512]
    d["cdT"] = np.ascontiguousarray(
        cd_sl.reshape(S, 8, 128, 3).transpose(0, 2, 1, 3))      # [S,128,8,3]
    d["lidT"] = np.ascontiguousarray(
        lid_sl.reshape(S, 8, 128).transpose(0, 2, 1)).astype(BF)  # [S,128,8]

    nm = np.zeros((NPAD, 70), np.float32)
    nm[0:NPC] = nfi[lo:hi]
    d["node_nm"] = nm.reshape(NBLK, 128, 70).transpose(1, 0, 2).reshape(128, NBLK * 70).copy()
    ic = np.ones(NPAD, np.float32)
    ic[0:NPC] = invcnt_all[lo:hi]
    d["invcnt"] = ic.reshape(NBLK, 128).T.copy()                # [128, NBLK]
    nl = np.zeros((64, NPAD), np.float32)
    nl[:, 0:NPC] = nfi[lo:hi, 6:70].T
    d["nfT_local"] = nl
    return d


def _build_program(CPB, S):
    NCH = S * 8
    nc = bacc.Bacc("TRN2", target_bir_lowering=False, debug=False,
                   enable_asserts=False, num_devices=NCORES)

    def din(name, shape, dt=f32):
        return nc.dram_tensor(name, list(shape), dt, kind="ExternalInput").ap()

    nfsT_d = din("nfsT", [S, 128, 512], bf16)
    nfeT_d = din("nfeT", [S, 128, 512], bf16)
    efcdn_d = din("efcdn", [S, 34, 512], bf16)
    cdT_d = din("cdT", [S, 128, 8, 3])
    lidT_d = din("lidT", [S, 128, 8], bf16)
    invcnt_d = din("invcnt", [128, NBLK])
    node_nm_d = din("node_nm", [128, NBLK * 70])
    nfT_loc_d = din("nfT_local", [64, NPAD])
    wd = {n: din(n, WSHAPES[n], bf16 if n in BF_W else f32) for n in WSHAPES}
    out_d = nc.dram_tensor("out", [NPAD, 70], f32, kind="ExternalOutput").ap()

    with tile.TileContext(nc) as tc, contextlib.ExitStack() as ctx:
        wpool = ctx.enter_context(tc.tile_pool(name="w", bufs=1))
        wt = {}
        for n in WSHAPES:
            t = wpool.tile(WSHAPES[n], bf16 if n in BF_W else f32, name=f"wt_{n}")
            nc.sync.dma_start(t[:], wd[n][:])
            wt[n] = t
        iota32 = wpool.tile([128, 128], i32, name="iota32")
        nc.gpsimd.iota(iota32[:], pattern=[[1, 128]], base=0, channel_multiplier=0)
        iota = wpool.tile([128, 128], bf16, name="iota")
        nc.vector.tensor_copy(iota[:], iota32[:])
        node_nm = wpool.tile([128, NBLK * 70], f32, name="node_nm")
        nc.sync.dma_start(node_nm[:], node_nm_d[:])
        invcnt = wpool.tile([128, NBLK], f32, name="invcnt")
        nc.sync.dma_start(invcnt[:], invcnt_d[:])
        nfT_loc = wpool.tile([64, NPAD], f32, name="nfT_loc")
        nc.sync.dma_start(nfT_loc[:], nfT_loc_d[:])
        vscale = wpool.tile([128, NBLK], f32, name="vscale")
        aggsb = wpool.tile([128, NBLK * 67], f32, name="aggsb")  # node-major [n, blk*67]

        # ---------- Phase B: velocity MLP -> vscale [128, NBLK] ----------
        with tc.tile_pool(name="pb", bufs=2) as pb, \
             tc.tile_pool(name="pbp", bufs=2, space="PSUM") as pbp:
            tiles = [(j * 512, 512) for j in range(NPAD // 512)]
            if NPAD % 512:
                tiles.append((NPAD // 512 * 512, NPAD % 512))
            for (o, L) in tiles:
                vps = pbp.tile([64, L], f32, name=f"vps{o}", tag="vps")
                nc.tensor.matmul(vps[:], wt["wv1"][:], nfT_loc[:, o:o + L])
                vh = pb.tile([64, L], f32, name=f"vh{o}", tag="vh")
                nc.scalar.activation(vh[:], vps[:], AF.Silu, bias=wt["bv1c"][:])
                sps = pbp.tile([1, L], f32, name=f"sps{o}", tag="sps")
                nc.tensor.matmul(sps[:], wt["wv2"][:], vh[:])
                vsc = pb.tile([1, L], f32, name=f"vsc{o}", tag="vsc")
                nc.scalar.activation(vsc[:], sps[:], AF.Identity, bias=wt["bv2c"][:])
                for k in range(L // 128):
                    tp = pbp.tile([128, 1], f32, name=f"tp{o}_{k}", tag="tp")
                    nc.tensor.transpose(tp[:], vsc[:, k * 128:(k + 1) * 128],
                                        wt["eye64"][0:1, 0:1])
                    nc.vector.tensor_copy(vscale[:, o // 128 + k:o // 128 + k + 1], tp[:])

        # ---------- Edge sweep ----------
        with tc.tile_pool(name="pin", bufs=4) as pin, \
             tc.tile_pool(name="pmid", bufs=3) as pmid, \
             tc.tile_pool(name="px", bufs=2, space="PSUM") as px, \
             tc.tile_pool(name="pm", bufs=2, space="PSUM") as pm, \
             tc.tile_pool(name="pst", bufs=2, space="PSUM") as pst, \
             tc.tile_pool(name="pagg", bufs=2, space="PSUM") as pagg:
            aggN = None
            for s in range(S):
                nfs = pin.tile([128, 512], bf16, name=f"nfs{s}", tag="nfs")
                nc.sync.dma_start(nfs[:], nfsT_d[s])
                nfe = pin.tile([128, 512], bf16, name=f"nfe{s}", tag="nfe")
                nc.sync.dma_start(nfe[:], nfeT_d[s])
                eft = pin.tile([34, 512], bf16, name=f"eft{s}", tag="eft")
                nc.sync.dma_start(eft[:], efcdn_d[s])
                cdt = pin.tile([128, 8, 3], f32, name=f"cdt{s}", tag="cdt")
                nc.sync.dma_start(cdt[:], cdT_d[s])
                lidt = pin.tile([128, 8], bf16, name=f"lidt{s}", tag="lidt")
                nc.sync.dma_start(lidt[:], lidT_d[s])

                x1 = px.tile([128, 512], f32, name=f"x1{s}", tag="x1")
                nc.tensor.matmul(x1[:], wt["w1s"][:], nfs[:], start=True, stop=False)
                nc.tensor.matmul(x1[:], wt["w1e"][:], nfe[:], start=False, stop=False)
                nc.tensor.matmul(x1[:], wt["wefcdn"][:], eft[:], start=False, stop=True)
                h1 = pmid.tile([128, 512], bf16, name=f"h1{s}", tag="h1")
                nc.scalar.activation(h1[:], x1[:], AF.Silu, bias=wt["be1s"][:])
                mp = pm.tile([128, 512], f32, name=f"mp{s}", tag="mm2")
                nc.tensor.matmul(mp[:], wt["wde2"][:], h1[:])
                # stacked [msg_g | ch_g] tiles per group
                stk = [pmid.tile([128, 512], bf16, name=f"stk{s}_{g}", tag=f"stk{g}")
                       for g in range(2)]
                for g in range(2):
                    nc.scalar.activation(stk[g][0:64, :], mp[g * 64:(g + 1) * 64, :],
                                         AF.Silu, bias=wt["be2c"][:])
                for g in range(2):
                    cp = pm.tile([64, 512], f32, name=f"cp{s}_{g}", tag="mm2")
                    nc.tensor.matmul(cp[:], wt["wdc1"][:], stk[g][0:64, :])
                    nc.scalar.activation(stk[g][64:128, :], cp[:],
                                         AF.Silu, bias=wt["bc1c"][:])

                oht = pmid.tile([128, 8, 128], bf16, name=f"oht{s}", tag="oht")
                nc.vector.tensor_tensor(
                    oht[:], iota[:].unsqueeze(1).broadcast_to([128, 8, 128]),
                    lidt[:].unsqueeze(2).broadcast_to([128, 8, 128]), OP.is_equal)
                rgc = pmid.tile([128, 8, 67], bf16, name=f"rgc{s}", tag="rgc")
                for g in range(2):
                    st = pst.tile([128, 4, 66], f32, name=f"st{s}_{g}", tag="st")
                    for c4 in range(4):
                        cc = slice(c4 * 128, (c4 + 1) * 128)
                        nc.tensor.matmul(st[:, c4, :], stk[g][:, cc], wt["raw2c"][:],
                                         start=True, stop=True)
                    tnh = pmid.tile([128, 4], f32, name=f"tnh{s}_{g}", tag="tnh")
                    nc.scalar.activation(tnh[:], st[:, :, 64:65].squeeze(2),
                                         AF.Tanh, bias=wt["bih"][:], scale=0.5)
                    gate = pmid.tile([128, 4], f32, name=f"gt{s}_{g}", tag="gate")
                    nc.vector.tensor_scalar(out=gate[:], in0=tnh[:], scalar1=1.0,
                                            scalar2=0.5, op0=OP.add, op1=OP.mult)
                    gsl = slice(g * 4, g * 4 + 4)
                    nc.vector.tensor_tensor(
                        rgc[:, gsl, 0:64], st[:, :, 0:64],
                        gate[:].unsqueeze(2).broadcast_to([128, 4, 64]), OP.mult)
                    nc.vector.tensor_tensor(
                        rgc[:, gsl, 64:67], cdt[:, gsl, :],
                        st[:, :, 65:66].broadcast_to([128, 4, 3]), OP.mult)

                for k in range(8):
                    gc = s * 8 + k
                    vb = gc // CPB
                    if vb >= NBLK:
                        continue
                    pos = gc % CPB
                    if pos == 0:
                        aggN = pagg.tile([128, 128], f32, name=f"agg{vb}", tag="agg")
                    nc.tensor.matmul(aggN[:, 0:67], oht[:, k, :], rgc[:, k, :],
                                     start=(pos == 0), stop=(pos == CPB - 1),
                                     skip_group_check=True)
                    if pos == CPB - 1:
                        nc.vector.tensor_copy(aggsb[:, vb * 67:(vb + 1) * 67],
                                              aggN[:, 0:67])

        # ---------- Phase C: node update + output ----------
        with tc.tile_pool(name="pc", bufs=3) as pc, \
             tc.tile_pool(name="pcp", bufs=2, space="PSUM") as pcp:
            b0 = 0
            while b0 < NBLK:
                BB = min(4, NBLK - b0)
                L = BB * 128
                xnT = pc.tile([128, BB, 128], f32, name=f"xnT{b0}", tag="xnT")
                nc.vector.tensor_copy(
                    xnT[0:64, :, :],
                    nfT_loc[:, b0 * 128:b0 * 128 + L].rearrange(
                        "p (b n) -> p b n", b=BB))
                atp = pcp.tile([64, BB, 128], f32, name=f"atp{b0}", tag="atp")
                for j in range(BB):
                    nc.tensor.transpose(
                        atp[:, j, :],
                        aggsb[:, (b0 + j) * 67:(b0 + j) * 67 + 64],
                        wt["eye128"][:])
                nc.vector.tensor_copy(xnT[64:128, :, :], atp[:])
                n1 = pcp.tile([64, BB, 128], f32, name=f"n1{b0}", tag="n1")
                nc.tensor.matmul(n1[:].rearrange("p b n -> p (b n)"), wt["wn1"][:],
                                 xnT[:].rearrange("p b n -> p (b n)"))
                hn = pc.tile([64, BB, 128], f32, name=f"hn{b0}", tag="hn")
                nc.scalar.activation(hn[:].rearrange("p b n -> p (b n)"),
                                     n1[:].rearrange("p b n -> p (b n)"),
                                     AF.Silu, bias=wt["bn1c"][:])
                n2 = pcp.tile([64, BB, 128], f32, name=f"n2{b0}", tag="n2")
                nc.tensor.matmul(n2[:].rearrange("p b n -> p (b n)"), wt["wn2"][:],
                                 hn[:].rearrange("p b n -> p (b n)"))
                hn2 = pc.tile([64, BB, 128], f32, name=f"hn2{b0}", tag="hn2")
                nc.scalar.activation(hn2[:].rearrange("p b n -> p (b n)"),
                                     n2[:].rearrange("p b n -> p (b n)"),
                                     AF.Identity, bias=wt["bn2c"][:])
                ndel = pcp.tile([128, BB, 64], f32, name=f"ndel{b0}", tag="ndel")
                for j in range(BB):
                    nc.tensor.transpose(ndel[:, j, :], hn2[:, j, :], wt["eye64"][:])
                nmb = node_nm[:, b0 * 70:(b0 + BB) * 70].rearrange(
                    "p (b f) -> p b f", b=BB)
                t1 = pc.tile([128, BB, 3], f32, name=f"t1{b0}", tag="t1")
                nc.vector.tensor_tensor(
                    t1[:],
                    aggsb[:, b0 * 67:(b0 + BB) * 67].rearrange(
                        "p (b f) -> p b f", b=BB)[:, :, 64:67],
                    invcnt[:, b0:b0 + BB].unsqueeze(2).broadcast_to([128, BB, 3]),
                    OP.mult)
                t2 = pc.tile([128, BB, 3], f32, name=f"t2{b0}", tag="t2")
                nc.vector.tensor_tensor(
                    t2[:], nmb[:, :, 3:6],
                    vscale[:, b0:b0 + BB].unsqueeze(2).broadcast_to([128, BB, 3]),
                    OP.mult)
                t3 = pc.tile([128, BB, 3], f32, name=f"t3{b0}", tag="t3")
                nc.vector.tensor_tensor(t3[:], t1[:], t2[:], OP.add)
                ot = pc.tile([128, BB, 70], f32, name=f"ot{b0}", tag="ot")
                nc.vector.tensor_tensor(ot[:, :, 0:3], t3[:], nmb[:, :, 0:3], OP.add)
                nc.vector.tensor_copy(ot[:, :, 3:6], nmb[:, :, 3:6])
                nc.vector.tensor_tensor(ot[:, :, 6:70], nmb[:, :, 6:70], ndel[:],
                                        OP.add)
                nc.sync.dma_start(
                    out_d[b0 * 128:(b0 + BB) * 128, :].rearrange(
                        "(b p) f -> p b f", p=128),
                    ot[:])
                b0 += BB

    nc.compile()
    return nc


def kernel(**inputs):
    ei = np.asarray(inputs["edge_indices"])
    start = ei[0].astype(np.int64)
    end = ei[1].astype(np.int64)
    ef = _f(inputs["edge_features"])
    nfi = _f(inputs["node_features_input"])
    coords = nfi[:, 0:3]
    cd_all = coords[start] - coords[end]
    cdn_all = np.sqrt((cd_all ** 2).sum(1)).astype(np.float32)
    deg = np.bincount(start, minlength=N).astype(np.float32)
    invcnt_all = (1.0 / np.maximum(deg, 1.0)).astype(np.float32)

    # uniform chunks-per-block across all cores/blocks (SPMD program shape)
    core = start // NPC
    lblk = (start - core * NPC) >> 7
    bc = np.bincount(core * NBLK + lblk, minlength=NCORES * NBLK)
    CPB = int(np.ceil(bc.max() / 128.0))
    NCHR = NBLK * CPB
    NCH = (NCHR + 7) // 8 * 8
    S = NCH // 8

    w = _prep_weights(inputs)
    in_maps = []
    for c in range(NCORES):
        d = _prep_core(c, start, end, ef, nfi, cd_all, cdn_all, invcnt_all, CPB, S)
        d.update(w)
        in_maps.append(d)

    key = (CPB, S)
    if _cache.get("key") != key:
        _cache["nc"] = _build_program(CPB, S)
        _cache["key"] = key
    nc = _cache["nc"]
    _cache["in_maps"] = in_maps
    res = run_bass_kernel_spmd(nc, in_maps, list(range(NCORES)))
    out = np.empty((N, 70), np.float32)
    for c in range(NCORES):
        out[c * NPC:(c + 1) * NPC] = res.results[c]["out"][0:NPC]
    return out


# revision 13
# speedup vs baseline: 5.2571x; 1.2437x over previous
"""EquivariantGraphConvolution (EGNN layer) on 8 Trainium2 NeuronCores.

Strategy (v2 — streamed, gather-free)
-------------------------------------
Nodes are range-partitioned across the 8 cores (6250 each); every edge is owned
by the core that owns its *start* node, so per-start segment sums are
core-local and no collective is needed.

Per core, edges are sorted by 128-node start block and padded per block to a
uniform CPB chunks of 128 edges.  The host pre-gathers both endpoints' node
features per edge and stages them as sequentially-streamed feature-major bf16
tensors (plus edge features / dist / coords-diff / lane ids), so the device
does ZERO indirect DMA — the edge MLP is pure dense matmul work:

  x1[128,512] = W1s_bd.T@nfs + W1e_bd.T@nfe + Wef.T@efcdn   (2 edge groups
  feature-stacked on partitions), SiLU chains for message/coords nets, a merged
  transpose+gate+coordw matmul per 128-edge chunk (K=128: msg|coord stacked),
  and a one-hot matmul segment-sum per chunk with the one-hot stationary
  (out is node-major [128,67] = 64 msg-agg + 3 coord-agg).

Node updates (velocity/node MLPs, coordinate update) run on-chip afterwards.
"""
import sys
sys.path.insert(0, "/opt/trn_rl_repo")
import contextlib
import numpy as np
import ml_dtypes

import concourse.bass as bass
import concourse.bacc as bacc
import concourse.mybir as mybir
import concourse.tile as tile
from concourse.bass_utils import run_bass_kernel_spmd

f32 = mybir.dt.float32
bf16 = mybir.dt.bfloat16
i32 = mybir.dt.int32
AF = mybir.ActivationFunctionType
OP = mybir.AluOpType
BF = ml_dtypes.bfloat16

# ---- problem constants (hardcoded per contract) ----
N = 50000
E = 1_000_000
H = 64
EF = 16
NCORES = 8
NPC = N // NCORES          # 6250 nodes per core
NBLK = 49                  # 128-node blocks per core (49*128 = 6272 >= 6250)
NPAD = NBLK * 128          # 6272

_cache = {}


def _f(x):
    return np.ascontiguousarray(x, np.float32)


def _bd(W):
    """[64,64] -> [128,128] block diagonal."""
    out = np.zeros((128, 128), np.float32)
    out[0:64, 0:64] = W
    out[64:128, 64:128] = W
    return out


def _prep_weights(inp):
    """Small weight/constant tensors, identical on all cores."""
    W_e1 = _f(inp["W_e1"])           # [145, 64]
    w = {}
    w["w1s"] = _bd(W_e1[0:64]).astype(BF)
    w["w1e"] = _bd(W_e1[64:128]).astype(BF)
    wef = np.zeros((34, 128), np.float32)
    wef[0:16, 0:64] = W_e1[129:145]
    wef[16:17, 0:64] = W_e1[128:129]
    wef[17:33, 64:128] = W_e1[129:145]
    wef[33:34, 64:128] = W_e1[128:129]
    w["wefcdn"] = wef.astype(BF)
    w["wde2"] = _bd(_f(inp["W_e2"])).astype(BF)
    w["wdc1"] = _bd(_f(inp["W_c1"])).astype(BF)
    W_i = _f(inp["W_i"]); W_c2 = _f(inp["W_c2"])
    ra = np.zeros((128, 65), np.float32)
    ra[0:64, 0:64] = np.eye(64); ra[0:64, 64:65] = W_i
    ra[64:128, 0:64] = np.eye(64); ra[64:128, 64:65] = W_i
    w["raw2"] = ra.astype(BF)
    wc2r = np.zeros((128, 1), np.float32)
    wc2r[0:64] = W_c2; wc2r[64:128] = W_c2
    w["wc2r"] = wc2r.astype(BF)
    w["wn1"] = _f(inp["W_n1"])       # [128, 64]
    w["wn2"] = _f(inp["W_n2"])       # [64, 64]
    w["wv1"] = _f(inp["W_v1"])       # [64, 64]
    w["wv2"] = _f(inp["W_v2"])       # [64, 1]
    w["eye64"] = np.eye(64, dtype=np.float32)
    w["eye128"] = np.eye(128, dtype=np.float32)
    b_e1 = _f(inp["b_e1"]); b_e2 = _f(inp["b_e2"]); b_c1 = _f(inp["b_c1"])
    w["be1s"] = np.concatenate([b_e1, b_e1]).reshape(128, 1)
    w["be2s"] = np.concatenate([b_e2, b_e2]).reshape(128, 1)
    w["bc1s"] = np.concatenate([b_c1, b_c1]).reshape(128, 1)
    w["bih"] = np.full((128, 1), 0.5 * float(np.asarray(inp["b_i"]).ravel()[0]), np.float32)
    w["bn1c"] = _f(inp["b_n1"]).reshape(64, 1)
    w["bn2c"] = _f(inp["b_n2"]).reshape(64, 1)
    w["bv1c"] = _f(inp["b_v1"]).reshape(64, 1)
    w["bv2c"] = np.full((1, 1), float(np.asarray(inp["b_v2"]).ravel()[0]), np.float32)
    return w


WSHAPES = {"w1s": [128, 128], "w1e": [128, 128], "wefcdn": [34, 128],
           "wde2": [128, 128], "wdc1": [128, 128], "raw2": [128, 65],
           "wc2r": [128, 1],
           "wn1": [128, 64], "wn2": [64, 64], "wv1": [64, 64],
           "wv2": [64, 1], "eye64": [64, 64], "eye128": [128, 128],
           "be1s": [128, 1], "be2s": [128, 1], "bc1s": [128, 1],
           "bih": [128, 1], "bn1c": [64, 1], "bn2c": [64, 1],
           "bv1c": [64, 1], "bv2c": [1, 1]}
BF_W = ("w1s", "w1e", "wefcdn", "wde2", "wdc1", "raw2", "wc2r")


def _prep_core(c, start, end, ef, nfi, cd_all, cdn_all, invcnt_all, CPB, S):
    """Per-core staged edge streams (sorted by start block, block-padded)."""
    NCH = S * 8
    NSLOT = NCH * 128
    lo, hi = c * NPC, (c + 1) * NPC
    sel = (start >= lo) & (start < hi)
    eo = np.nonzero(sel)[0]
    s_loc = (start[eo] - lo).astype(np.int64)
    blk = s_loc >> 7
    order = np.argsort(blk, kind="stable")
    eo = eo[order]; s_loc = s_loc[order]; blk = blk[order]
    counts = np.bincount(blk, minlength=NBLK)
    if counts.max() > CPB * 128:
        raise RuntimeError(f"block overflow: {counts.max()} > {CPB * 128}")
    starts = np.zeros(NBLK, np.int64)
    starts[1:] = np.cumsum(counts)[:-1]
    within = np.arange(len(eo)) - starts[blk]
    slots = blk * (CPB * 128) + within

    nf64 = nfi[:, 6:70]
    nfs_sl = np.zeros((NSLOT, 64), np.float32)
    nfe_sl = np.zeros((NSLOT, 64), np.float32)
    ef_sl = np.zeros((NSLOT, EF), np.float32)
    cdn_sl = np.zeros(NSLOT, np.float32)
    cd_sl = np.zeros((NSLOT, 3), np.float32)
    lid_sl = np.full(NSLOT, -1.0, np.float32)
    nfs_sl[slots] = nf64[start[eo]]
    nfe_sl[slots] = nf64[end[eo]]
    ef_sl[slots] = ef[eo]
    cdn_sl[slots] = cdn_all[eo]
    cd_sl[slots] = cd_all[eo]
    lid_sl[slots] = (s_loc & 127).astype(np.float32)

    d = {}
    # feature-major, 2 edge groups of 512 stacked on partitions;
    # nfse packs [nfs | nfe] along the free dim -> one DMA per supertile
    nfse = np.empty((S, 128, 1024), BF)
    v = nfs_sl.reshape(S, 2, 512, 64).transpose(0, 1, 3, 2)
    nfse[:, :, 0:512] = v.reshape(S, 128, 512)
    v = nfe_sl.reshape(S, 2, 512, 64).transpose(0, 1, 3, 2)
    nfse[:, :, 512:1024] = v.reshape(S, 128, 512)
    d["nfse"] = nfse
    eft = ef_sl.reshape(S, 2, 512, EF).transpose(0, 1, 3, 2)   # [S,2,16,512]
    cdnr = cdn_sl.reshape(S, 2, 512)
    d["efcdn"] = np.concatenate(
        [eft[:, 0], cdnr[:, 0][:, None, :], eft[:, 1], cdnr[:, 1][:, None, :]],
        axis=1).astype(BF)                                      # [S,34,512]
    cdlid = np.empty((S, 128, 8, 4), BF)
    cdlid[:, :, :, 0:3] = cd_sl.reshape(S, 8, 128, 3).transpose(0, 2, 1, 3)
    cdlid[:, :, :, 3] = lid_sl.reshape(S, 8, 128).transpose(0, 2, 1)
    d["cdlid"] = cdlid

    nm = np.zeros((NPAD, 70), np.float32)
    nm[0:NPC] = nfi[lo:hi]
    d["node_nm"] = nm.reshape(NBLK, 128, 70).transpose(1, 0, 2).reshape(128, NBLK * 70).copy()
    ic = np.ones(NPAD, np.float32)
    ic[0:NPC] = invcnt_all[lo:hi]
    d["invcnt"] = ic.reshape(NBLK, 128).T.copy()                # [128, NBLK]
    nl = np.zeros((64, NPAD), np.float32)
    nl[:, 0:NPC] = nfi[lo:hi, 6:70].T
    d["nfT_local"] = nl
    return d


def _build_program(CPB, S):
    NCH = S * 8
    nc = bacc.Bacc("TRN2", target_bir_lowering=False, debug=False,
                   enable_asserts=False, num_devices=NCORES)

    def din(name, shape, dt=f32):
        return nc.dram_tensor(name, list(shape), dt, kind="ExternalInput").ap()

    nfse_d = din("nfse", [S, 128, 1024], bf16)
    efcdn_d = din("efcdn", [S, 34, 512], bf16)
    cdlid_d = din("cdlid", [S, 128, 8, 4], bf16)
    invcnt_d = din("invcnt", [128, NBLK])
    node_nm_d = din("node_nm", [128, NBLK * 70])
    nfT_loc_d = din("nfT_local", [64, NPAD])
    wd = {n: din(n, WSHAPES[n], bf16 if n in BF_W else f32) for n in WSHAPES}
    out_d = nc.dram_tensor("out", [NPAD, 70], f32, kind="ExternalOutput").ap()

    with tile.TileContext(nc) as tc, contextlib.ExitStack() as ctx:
        wpool = ctx.enter_context(tc.tile_pool(name="w", bufs=1))
        wt = {}
        for n in WSHAPES:
            t = wpool.tile(WSHAPES[n], bf16 if n in BF_W else f32, name=f"wt_{n}")
            nc.sync.dma_start(t[:], wd[n][:])
            wt[n] = t
        iota32 = wpool.tile([128, 128], i32, name="iota32")
        nc.gpsimd.iota(iota32[:], pattern=[[1, 128]], base=0, channel_multiplier=0)
        iota = wpool.tile([128, 128], bf16, name="iota")
        nc.vector.tensor_copy(iota[:], iota32[:])
        node_nm = wpool.tile([128, NBLK * 70], f32, name="node_nm")
        nc.sync.dma_start(node_nm[:], node_nm_d[:])
        invcnt = wpool.tile([128, NBLK], f32, name="invcnt")
        nc.sync.dma_start(invcnt[:], invcnt_d[:])
        nfT_loc = wpool.tile([64, NPAD], f32, name="nfT_loc")
        nc.sync.dma_start(nfT_loc[:], nfT_loc_d[:])
        vscale = wpool.tile([128, NBLK], f32, name="vscale")
        aggsb = wpool.tile([128, NBLK * 67], f32, name="aggsb")  # node-major [n, blk*67]

        # ---------- Phase B: velocity MLP -> vscale [128, NBLK] ----------
        with tc.tile_pool(name="pb", bufs=2) as pb, \
             tc.tile_pool(name="pbp", bufs=2, space="PSUM") as pbp:
            tiles = [(j * 512, 512) for j in range(NPAD // 512)]
            if NPAD % 512:
                tiles.append((NPAD // 512 * 512, NPAD % 512))
            for (o, L) in tiles:
                vps = pbp.tile([64, L], f32, name=f"vps{o}", tag="vps")
                nc.tensor.matmul(vps[:], wt["wv1"][:], nfT_loc[:, o:o + L])
                vh = pb.tile([64, L], f32, name=f"vh{o}", tag="vh")
                nc.scalar.activation(vh[:], vps[:], AF.Silu, bias=wt["bv1c"][:])
                sps = pbp.tile([1, L], f32, name=f"sps{o}", tag="sps")
                nc.tensor.matmul(sps[:], wt["wv2"][:], vh[:])
                vsc = pb.tile([1, L], f32, name=f"vsc{o}", tag="vsc")
                nc.scalar.activation(vsc[:], sps[:], AF.Identity, bias=wt["bv2c"][:])
                for k in range(L // 128):
                    tp = pbp.tile([128, 1], f32, name=f"tp{o}_{k}", tag="tp")
                    nc.tensor.transpose(tp[:], vsc[:, k * 128:(k + 1) * 128],
                                        wt["eye64"][0:1, 0:1])
                    nc.vector.tensor_copy(vscale[:, o // 128 + k:o // 128 + k + 1], tp[:])

        # ---------- Edge sweep ----------
        with tc.tile_pool(name="pin", bufs=6) as pin, \
             tc.tile_pool(name="pmid", bufs=4) as pmid, \
             tc.tile_pool(name="px", bufs=2, space="PSUM") as px, \
             tc.tile_pool(name="pm", bufs=2, space="PSUM") as pm, \
             tc.tile_pool(name="pst", bufs=2, space="PSUM") as pst, \
             tc.tile_pool(name="pagg", bufs=2, space="PSUM") as pagg:
            aggN = None
            for s in range(S):
                nfse = pin.tile([128, 1024], bf16, name=f"nfse{s}", tag="nfse")
                nc.sync.dma_start(nfse[:], nfse_d[s])
                eft = pin.tile([34, 512], bf16, name=f"eft{s}", tag="eft")
                nc.sync.dma_start(eft[:], efcdn_d[s])
                cdlid = pin.tile([128, 8, 4], bf16, name=f"cdlid{s}", tag="cdlid")
                nc.sync.dma_start(cdlid[:], cdlid_d[s])

                oht = pmid.tile([128, 8, 128], bf16, name=f"oht{s}", tag="oht")
                nc.vector.tensor_tensor(
                    oht[:], iota[:].unsqueeze(1).broadcast_to([128, 8, 128]),
                    cdlid[:, :, 3:4].broadcast_to([128, 8, 128]), OP.is_equal)

                x1 = px.tile([128, 512], f32, name=f"x1{s}", tag="x1")
                nc.tensor.matmul(x1[:], wt["w1s"][:], nfse[:, 0:512],
                                 start=True, stop=False)
                nc.tensor.matmul(x1[:], wt["w1e"][:], nfse[:, 512:1024],
                                 start=False, stop=False)
                nc.tensor.matmul(x1[:], wt["wefcdn"][:], eft[:], start=False, stop=True)
                h1 = pmid.tile([128, 512], bf16, name=f"h1{s}", tag="h1")
                nc.scalar.activation(h1[:], x1[:], AF.Silu, bias=wt["be1s"][:])
                mp = pm.tile([128, 512], f32, name=f"mp{s}", tag="mm2")
                nc.tensor.matmul(mp[:], wt["wde2"][:], h1[:])
                msgT = pmid.tile([128, 512], bf16, name=f"msgT{s}", tag="msgT")
                nc.scalar.activation(msgT[:], mp[:], AF.Silu, bias=wt["be2s"][:])
                cp = pm.tile([128, 512], f32, name=f"cp{s}", tag="mm2")
                nc.tensor.matmul(cp[:], wt["wdc1"][:], msgT[:])
                chT = pmid.tile([128, 512], bf16, name=f"chT{s}", tag="chT")
                nc.scalar.activation(chT[:], cp[:], AF.Silu, bias=wt["bc1s"][:])

                rgc = pmid.tile([128, 8, 67], bf16, name=f"rgc{s}", tag="rgc")
                for g in range(2):
                    rows = slice(g * 64, g * 64 + 64)
                    st = pst.tile([128, 4, 66], f32, name=f"st{s}_{g}", tag="st")
                    for c4 in range(4):
                        cc = slice(c4 * 128, (c4 + 1) * 128)
                        nc.tensor.matmul(st[:, c4, 0:65], msgT[rows, cc],
                                         wt["raw2"][rows, :], start=True, stop=True)
                        nc.tensor.matmul(st[:, c4, 65:66], chT[rows, cc],
                                         wt["wc2r"][rows, :], start=True, stop=True)
                    tnh = pmid.tile([128, 4], f32, name=f"tnh{s}_{g}", tag="tnh")
                    nc.scalar.activation(tnh[:], st[:, :, 64:65].squeeze(2),
                                         AF.Tanh, bias=wt["bih"][:], scale=0.5)
                    gate = pmid.tile([128, 4], f32, name=f"gt{s}_{g}", tag="gate")
                    nc.vector.tensor_scalar(out=gate[:], in0=tnh[:], scalar1=1.0,
                                            scalar2=0.5, op0=OP.add, op1=OP.mult)
                    gsl = slice(g * 4, g * 4 + 4)
                    nc.vector.tensor_tensor(
                        rgc[:, gsl, 0:64], st[:, :, 0:64],
                        gate[:].unsqueeze(2).broadcast_to([128, 4, 64]), OP.mult)
                    nc.vector.tensor_tensor(
                        rgc[:, gsl, 64:67], cdlid[:, gsl, 0:3],
                        st[:, :, 65:66].broadcast_to([128, 4, 3]), OP.mult)

                for k in range(8):
                    gc = s * 8 + k
                    vb = gc // CPB
                    if vb >= NBLK:
                        continue
                    pos = gc % CPB
                    if pos == 0:
                        aggN = pagg.tile([128, 128], f32, name=f"agg{vb}", tag="agg")
                    nc.tensor.matmul(aggN[:, 0:67], oht[:, k, :], rgc[:, k, :],
                                     start=(pos == 0), stop=(pos == CPB - 1),
                                     skip_group_check=True)
                    if pos == CPB - 1:
                        nc.vector.tensor_copy(aggsb[:, vb * 67:(vb + 1) * 67],
                                              aggN[:, 0:67])

        # ---------- Phase C: node update + output ----------
        with tc.tile_pool(name="pc", bufs=3) as pc, \
             tc.tile_pool(name="pcp", bufs=2, space="PSUM") as pcp:
            b0 = 0
            while b0 < NBLK:
                BB = min(4, NBLK - b0)
                L = BB * 128
                xnT = pc.tile([128, BB, 128], f32, name=f"xnT{b0}", tag="xnT")
                nc.vector.tensor_copy(
                    xnT[0:64, :, :],
                    nfT_loc[:, b0 * 128:b0 * 128 + L].rearrange(
                        "p (b n) -> p b n", b=BB))
                atp = pcp.tile([64, BB, 128], f32, name=f"atp{b0}", tag="atp")
                for j in range(BB):
                    nc.tensor.transpose(
                        atp[:, j, :],
                        aggsb[:, (b0 + j) * 67:(b0 + j) * 67 + 64],
                        wt["eye128"][:])
                nc.vector.tensor_copy(xnT[64:128, :, :], atp[:])
                n1 = pcp.tile([64, BB, 128], f32, name=f"n1{b0}", tag="n1")
                nc.tensor.matmul(n1[:].rearrange("p b n -> p (b n)"), wt["wn1"][:],
                                 xnT[:].rearrange("p b n -> p (b n)"))
                hn = pc.tile([64, BB, 128], f32, name=f"hn{b0}", tag="hn")
                nc.scalar.activation(hn[:].rearrange("p b n -> p (b n)"),
                                     n1[:].rearrange("p b n -> p (b n)"),
                                     AF.Silu, bias=wt["bn1c"][:])
                n2 = pcp.tile([64, BB, 128], f32, name=f"n2{b0}", tag="n2")
                nc.tensor.matmul(n2[:].rearrange("p b n -> p (b n)"), wt["wn2"][:],
                                 hn[:].rearrange("p b n -> p (b n)"))
                hn2 = pc.tile([64, BB, 128], f32, name=f"hn2{b0}", tag="hn2")
                nc.scalar.activation(hn2[:].rearrange("p b n -> p (b n)"),
                                     n2[:].rearrange("p b n -> p (b n)"),
                                     AF.Identity, bias=wt["bn2c"][:])
                ndel = pcp.tile([128, BB, 64], f32, name=f"ndel{b0}", tag="ndel")
                for j in range(BB):
                    nc.tensor.transpose(ndel[:, j, :], hn2[:, j, :], wt["eye64"][:])
                nmb = node_nm[:, b0 * 70:(b0 + BB) * 70].rearrange(
                    "p (b f) -> p b f", b=BB)
                t1 = pc.tile([128, BB, 3], f32, name=f"t1{b0}", tag="t1")
                nc.vector.tensor_tensor(
                    t1[:],
                    aggsb[:, b0 * 67:(b0 + BB) * 67].rearrange(
                        "p (b f) -> p b f", b=BB)[:, :, 64:67],
                    invcnt[:, b0:b0 + BB].unsqueeze(2).broadcast_to([128, BB, 3]),
                    OP.mult)
                t2 = pc.tile([128, BB, 3], f32, name=f"t2{b0}", tag="t2")
                nc.vector.tensor_tensor(
                    t2[:], nmb[:, :, 3:6],
                    vscale[:, b0:b0 + BB].unsqueeze(2).broadcast_to([128, BB, 3]),
                    OP.mult)
                t3 = pc.tile([128, BB, 3], f32, name=f"t3{b0}", tag="t3")
                nc.vector.tensor_tensor(t3[:], t1[:], t2[:], OP.add)
                ot = pc.tile([128, BB, 70], f32, name=f"ot{b0}", tag="ot")
                nc.vector.tensor_tensor(ot[:, :, 0:3], t3[:], nmb[:, :, 0:3], OP.add)
                nc.vector.tensor_copy(ot[:, :, 3:6], nmb[:, :, 3:6])
                nc.vector.tensor_tensor(ot[:, :, 6:70], nmb[:, :, 6:70], ndel[:],
                                        OP.add)
                nc.sync.dma_start(
                    out_d[b0 * 128:(b0 + BB) * 128, :].rearrange(
                        "(b p) f -> p b f", p=128),
                    ot[:])
                b0 += BB

    nc.compile()
    return nc


def kernel(**inputs):
    ei = np.asarray(inputs["edge_indices"])
    start = ei[0].astype(np.int64)
    end = ei[1].astype(np.int64)
    ef = _f(inputs["edge_features"])
    nfi = _f(inputs["node_features_input"])
    coords = nfi[:, 0:3]
    cd_all = coords[start] - coords[end]
    cdn_all = np.sqrt((cd_all ** 2).sum(1)).astype(np.float32)
    deg = np.bincount(start, minlength=N).astype(np.float32)
    invcnt_all = (1.0 / np.maximum(deg, 1.0)).astype(np.float32)

    # uniform chunks-per-block across all cores/blocks (SPMD program shape)
    core = start // NPC
    lblk = (start - core * NPC) >> 7
    bc = np.bincount(core * NBLK + lblk, minlength=NCORES * NBLK)
    CPB = int(np.ceil(bc.max() / 128.0))
    NCHR = NBLK * CPB
    NCH = (NCHR + 7) // 8 * 8
    S = NCH // 8

    w = _prep_weights(inputs)
    in_maps = []
    for c in range(NCORES):
        d = _prep_core(c, start, end, ef, nfi, cd_all, cdn_all, invcnt_all, CPB, S)
        d.update(w)
        in_maps.append(d)

    key = (CPB, S)
    if _cache.get("key") != key:
        _cache["nc"] = _build_program(CPB, S)
        _cache["key"] = key
    nc = _cache["nc"]
    _cache["in_maps"] = in_maps
    res = run_bass_kernel_spmd(nc, in_maps, list(range(NCORES)))
    out = np.empty((N, 70), np.float32)
    for c in range(NCORES):
        out[c * NPC:(c + 1) * NPC] = res.results[c]["out"][0:NPC]
    return out
